# revision 14
# baseline (speedup 1.0000x reference)
"""TRN2 Bass kernel for nn_MultiHeadHyperedgeAttention.

Pipeline (8 NeuronCores, hyperedge-sharded, no collectives):
  host: sort edges by hyperedge; bin-pack segments into bins of <=64 slots
        with <=128 edges per node-shard (4 shards of 25000 rows so gather
        indices fit int16); build per-edge slot/weight tables.
  dev:  dma_gather x rows per (chunk, shard) on 4 SWDGE queues ->
        per-bin matmul G^T @ M accumulated over shards in PSUM
        (M = scaled one-hot built on-device via tensor_scalar) ->
        batched per-head MLP (3 matmuls + ACT ops) over all slots ->
        one f32 per slot.
  host: scatter slot outputs back to the [50000] output.
"""
import numpy as np

import concourse.bass as bass
import concourse.tile as tile
from concourse import bacc, mybir
from concourse.library_config import mlp as mlp_lib
from concourse.bass_utils import run_bass_kernel_spmd

NUM_NODES = 100000
NUM_HYPEREDGES = 50000
IN_DIM = 128
N_CORES = 8
N_SHARDS = 4
SHARD = NUM_NODES // N_SHARDS      # 25000 rows -> int16-safe gather indices
SLOTS = 64                         # segment slots per bin
BINCAP = 128                       # per-shard edge capacity per bin
KB = 16                            # bins per gather chunk
NIDX = KB * BINCAP                 # indices per dma_gather call
PAD_SLOT = 999.0
P = 128
D = IN_DIM
F32 = mybir.dt.float32
F16 = mybir.dt.float16
F8 = mybir.dt.float8e4
I16 = mybir.dt.int16
AF = mybir.ActivationFunctionType
OP = mybir.AluOpType
SIG_LO = 1.0 / (1.0 + np.exp(5.0))
SIG_HI = 1.0 / (1.0 + np.exp(-5.0))

# fp8 rows halve the gather descriptor size (128B vs 256B); x rows are
# stored padded to a 256B stride (the gather ucode encodes src stride in
# 256B units)
GATHER_FP8 = True
XPAD = 256                         # fp8 elements per padded x row
# fp8 UNSCALED one-hot (exact 1.0) halves m4 traffic; the 1/count scale
# moves past the (b1==0) ReLU onto the [8, slots] logits as a DVE multiply.
# Only valid when b1 == 0 (kernel() checks and falls back).
M4_FP8 = True


def _dma_gather_small(g, out_ap, in_ap, idxs_ap, num_idxs, elem_size,
                      elem_step, single_packet=False, queue_num=0):
    """dma_gather clone with the elem_size_bytes%256 assert relaxed
    (128B elements; src stride must still be a 256B multiple)."""
    from concourse import ap_utils
    g._assert_queue_num(queue_num)
    assert idxs_ap.dtype == mybir.dt.int16
    assert in_ap.dtype == out_ap.dtype
    assert ap_utils.ap_is_contiguous(in_ap.ap[1:])
    assert ap_utils.ap_is_contiguous(out_ap.ap[1:])
    assert ap_utils.ap_is_contiguous(idxs_ap.ap[1:])
    assert in_ap.ap[-1][1] == out_ap.ap[-1][1] == elem_size
    assert out_ap.ap[0][1] * out_ap.ap[1][1] == -(-num_idxs // 128) * 128
    assert in_ap.ap[0][0] == elem_step
    stride_bytes = elem_step * mybir.dt.size(in_ap.dtype)
    assert stride_bytes % 256 == 0
    stride_bytes_256 = stride_bytes // 256
    assert stride_bytes_256 < 256
    _in_ap = g.lower_ap_dma(in_ap, for_custom_bir_dma=True)
    _idxs_ap = g.lower_ap(idxs_ap)
    _out_ap = g.lower_ap(out_ap)
    return g.add_instruction(
        mybir.InstDMAGatherAnt(
            name=g.bass.get_next_instruction_name(),
            ins=[*_in_ap, _idxs_ap, g.lower_val_access(g.to_reg(num_idxs))],
            outs=[_out_ap],
            transpose=False,
            num_idxs=num_idxs,
            elem_size=elem_size,
            stride_bytes_256=stride_bytes_256,
            gen_mode=0,
            single_packet=single_packet,
            queue_num=queue_num,
            sbuf_tokens_per_rank=0,
            sbuf_free_dim_per_rank=0,
            sbuf_free_dim_pad_per_rank=0,
            sbuf_byte_offset=0,
        )
    )


# ---------------------------------------------------------------- host packing

def _pack(node_idx, hyperedge_idx):
    node_idx = np.asarray(node_idx, dtype=np.int64)
    hyperedge_idx = np.asarray(hyperedge_idx, dtype=np.int64)
    counts = np.bincount(hyperedge_idx, minlength=NUM_HYPEREDGES)
    inv_cnt = 1.0 / np.maximum(counts, 1).astype(np.float64)

    shard_of_edge = node_idx // SHARD
    order = np.lexsort((node_idx, shard_of_edge, hyperedge_idx))
    e_node = node_idx[order]
    e_shard = shard_of_edge[order]

    cnt_ss = np.zeros((NUM_HYPEREDGES, N_SHARDS), dtype=np.int64)
    np.add.at(cnt_ss, (hyperedge_idx, shard_of_edge), 1)
    seg_starts = np.zeros(NUM_HYPEREDGES + 1, dtype=np.int64)
    seg_starts[1:] = np.cumsum(counts)

    # segments whose per-shard edge count exceeds one bin go to the host
    # fallback path (never happens for the target distribution)
    fallback = np.where(cnt_ss.max(axis=1) > BINCAP)[0]
    fb = set(fallback.tolist())

    seg_per_core = NUM_HYPEREDGES // N_CORES
    cores = []
    for c in range(N_CORES):
        s0, s1 = c * seg_per_core, (c + 1) * seg_per_core
        bins, cur_segs = [], []
        cur_cnt = np.zeros(N_SHARDS, dtype=np.int64)
        for s in range(s0, s1):
            if s in fb:
                continue
            csm = cnt_ss[s]
            if cur_segs and (len(cur_segs) >= SLOTS or np.any(cur_cnt + csm > BINCAP)):
                bins.append((cur_segs, cur_cnt))
                cur_segs, cur_cnt = [], np.zeros(N_SHARDS, dtype=np.int64)
            cur_segs = cur_segs + [s]
            cur_cnt = cur_cnt + csm
        if cur_segs:
            bins.append((cur_segs, cur_cnt))
        cores.append(bins)

    nbins = max(len(b) for b in cores)
    nbins = -(-nbins // KB) * KB
    nchunks = nbins // KB

    idx16 = np.zeros((N_CORES, N_SHARDS, nbins, BINCAP), dtype=np.int16)
    slotf = np.full((N_CORES, nbins, BINCAP, N_SHARDS), PAD_SLOT, dtype=np.float32)
    wf = np.zeros((N_CORES, nbins, BINCAP, N_SHARDS), dtype=np.float32)
    out_map = np.full((N_CORES, nbins, SLOTS), -1, dtype=np.int64)

    for c in range(N_CORES):
        for b, (segs, _cnt) in enumerate(cores[c]):
            out_map[c, b, :len(segs)] = segs
            pos = np.zeros(N_SHARDS, dtype=np.int64)
            for sl, s in enumerate(segs):
                e0, e1 = seg_starts[s], seg_starts[s + 1]
                nodes = e_node[e0:e1]
                shards = e_shard[e0:e1]
                for sh in range(N_SHARDS):
                    msk = shards == sh
                    k = int(msk.sum())
                    if k == 0:
                        continue
                    p0 = pos[sh]
                    idx16[c, sh, b, p0:p0 + k] = (nodes[msk] - sh * SHARD).astype(np.int16)
                    slotf[c, b, p0:p0 + k, sh] = sl
                    wf[c, b, p0:p0 + k, sh] = inv_cnt[s]
                    pos[sh] += k
            # sort each shard's 128 positions by node id for HBM locality
            for sh in range(N_SHARDS):
                o = np.argsort(idx16[c, sh, b], kind="stable")
                idx16[c, sh, b] = idx16[c, sh, b][o]
                slotf[c, b, :, sh] = slotf[c, b, o, sh]
                wf[c, b, :, sh] = wf[c, b, o, sh]

    nidx = KB * BINCAP
    gidx = np.zeros((N_CORES, N_SHARDS, nchunks, P, nidx // 16), dtype=np.int16)
    for c in range(N_CORES):
        for sh in range(N_SHARDS):
            flat = idx16[c, sh].reshape(nchunks, nidx)
            # wrapped layout: idx i -> partition i%16 (tiled x8), col i//16
            w = flat.reshape(nchunks, nidx // 16, 16).transpose(0, 2, 1)
            gidx[c, sh] = np.tile(w, (1, 8, 1))

    meta = dict(nbins=nbins, nchunks=nchunks, nslots=nbins * SLOTS)
    return dict(gidx=gidx, slotf=slotf, wf=wf, out_map=out_map,
                fallback=fallback, meta=meta,
                inv_cnt=inv_cnt.astype(np.float32))


def _make_mlp_consts(W1, b1, W2, b2):
    W1 = np.asarray(W1, np.float32); b1 = np.asarray(b1, np.float32)
    W2 = np.asarray(W2, np.float32); b2 = np.asarray(b2, np.float32)
    H, Din, K = W1.shape
    w1cat = np.ascontiguousarray(W1.transpose(1, 0, 2).reshape(Din, H * K))
    w2blk = np.zeros((H * K, H), np.float32)
    for h in range(H):
        w2blk[h * K:(h + 1) * K, h] = W2[h]
    return dict(w1cat=w1cat, b1cat=b1.reshape(H * K, 1),
                w2blk=w2blk, b2col=b2.reshape(H, 1),
                meanw=np.full((H, 1), 0.9 / H, np.float32))


def _make_in_map(core, x, packed, consts, m4_fp8=False):
    m = packed["meta"]
    nchunks, nbins = m["nchunks"], m["nbins"]
    slotf, wf = packed["slotf"][core], packed["wf"][core]  # [nbins, 128, 4]
    iota = np.arange(SLOTS, dtype=np.float32)
    if m4_fp8:
        m4 = ((slotf[..., None] == iota) & (wf[..., None] > 0)).astype(
            mybir.dt.np(F8))
    else:
        m4 = ((slotf[..., None] == iota) * wf[..., None]).astype(np.float16)
    # [nbins, 128, 4, 64] -> [nchunks, 128, KB*4*64]
    m4 = m4.reshape(nchunks, KB, P, N_SHARDS * SLOTS).transpose(0, 2, 1, 3)
    m4 = np.ascontiguousarray(m4).reshape(nchunks, P, KB * N_SHARDS * SLOTS)
    im = {
        "gidx": packed["gidx"][core],
        "m4": m4,
        **consts,
    }
    if m4_fp8:
        om = packed["out_map"][core].reshape(-1)       # [nslots]
        ic = np.ones(om.shape, np.float32)
        v = om >= 0
        ic[v] = packed["inv_cnt"][om[v]]
        im["icnt8"] = np.ascontiguousarray(
            np.broadcast_to(ic[None, :], (8, ic.shape[0])).astype(np.float32))
    f8np = mybir.dt.np(F8)
    for s in range(N_SHARDS):
        xs = np.ascontiguousarray(x[s * SHARD:(s + 1) * SHARD])
        if GATHER_FP8:
            pad = np.zeros((SHARD, XPAD), dtype=f8np)
            pad[:, :D] = xs.astype(f8np)
            im[f"xs{s}"] = pad
        else:
            im[f"xs{s}"] = xs.astype(np.float16)
    return im


# ---------------------------------------------------------------- device kernel

def build_nc(nbins, nchunks, n_cores, mlp_chunk=512, repeat=1,
             timing_mode=False, m4_fp8=None):
    if m4_fp8 is None:
        m4_fp8 = M4_FP8
    nslots = nbins * SLOTS
    assert nchunks * KB == nbins and nslots % mlp_chunk == 0
    nc = bacc.Bacc("TRN2", target_bir_lowering=False, debug=False,
                   num_devices=n_cores, num_swdge_queues=4)
    # timing_mode: big constant tensors become Internal (garbage content,
    # identical instruction stream/timing) so per-exec axon input shipping
    # shrinks to the gidx tensor only
    kind = "Internal" if timing_mode else "ExternalInput"
    if GATHER_FP8:
        xs = [nc.dram_tensor(f"xs{s}", [SHARD, XPAD], F8, kind=kind).ap()
              for s in range(N_SHARDS)]
    else:
        xs = [nc.dram_tensor(f"xs{s}", [SHARD, D], F16, kind=kind).ap()
              for s in range(N_SHARDS)]
    gidx = nc.dram_tensor("gidx", [N_SHARDS, nchunks, P, NIDX // 16], I16,
                          kind="ExternalInput").ap()
    m4_d = nc.dram_tensor("m4", [nchunks, P, KB * N_SHARDS * SLOTS],
                          F8 if m4_fp8 else F16, kind=kind).ap()
    icnt_d = (nc.dram_tensor("icnt8", [8, nslots], F32, kind=kind).ap()
              if m4_fp8 else None)
    w1_d = nc.dram_tensor("w1cat", [D, 64], F32, kind=kind).ap()
    b1_d = nc.dram_tensor("b1cat", [64, 1], F32, kind=kind).ap()
    w2_d = nc.dram_tensor("w2blk", [64, 8], F32, kind=kind).ap()
    b2_d = nc.dram_tensor("b2col", [8, 1], F32, kind=kind).ap()
    mean_d = nc.dram_tensor("meanw", [8, 1], F32, kind=kind).ap()
    out_d = nc.dram_tensor("out", [1, nslots], F32, kind="ExternalOutput").ap()

    with tile.TileContext(nc) as tc:
        with (
            tc.tile_pool(name="consts", bufs=1) as cpool,
            tc.tile_pool(name="idx", bufs=12) as ipool,
            tc.tile_pool(name="g", bufs=10) as gpool,
            tc.tile_pool(name="m4w", bufs=3) as mpool,
            tc.tile_pool(name="feats", bufs=1) as fpool,
            tc.tile_pool(name="mlptmp", bufs=3) as tpool,
            tc.tile_pool(name="outp", bufs=1) as opool,
            tc.tile_pool(name="psf", bufs=3, space="PSUM") as psf,
            tc.tile_pool(name="psh", bufs=2, space="PSUM") as psh,
            tc.tile_pool(name="psa", bufs=1, space="PSUM") as psa,
            tc.tile_pool(name="pso", bufs=1, space="PSUM") as pso,
        ):
            nc.gpsimd.load_library(mlp_lib)
            w1_t = cpool.tile([D, 64], F32)
            nc.sync.dma_start(out=w1_t[:], in_=w1_d[:])
            b1_t = cpool.tile([64, 1], F32)
            nc.sync.dma_start(out=b1_t[:], in_=b1_d[:])
            w2_t = cpool.tile([64, 8], F32)
            nc.sync.dma_start(out=w2_t[:], in_=w2_d[:])
            b2_t = cpool.tile([8, 1], F32)
            nc.sync.dma_start(out=b2_t[:], in_=b2_d[:])
            mean_t = cpool.tile([8, 1], F32)
            nc.sync.dma_start(out=mean_t[:], in_=mean_d[:])
            if m4_fp8:
                icnt_t = cpool.tile([8, nslots], F32)
                nc.sync.dma_start(out=icnt_t[:], in_=icnt_d[:])

            featsT = fpool.tile([P, nslots], F32)
            out_sb = opool.tile([1, nslots], F32)

            for _r in range(repeat):
                for ch in range(nchunks):
                    gts = []
                    for s in range(N_SHARDS):
                        it = ipool.tile([P, NIDX // 16], I16, tag="idx")
                        nc.sync.dma_start(out=it[:], in_=gidx[s, ch])
                        if GATHER_FP8:
                            G = gpool.tile([P, KB, D], F8, tag="G")
                            _dma_gather_small(nc.gpsimd, G[:],
                                              xs[s][:, 0:D], it[:], NIDX, D,
                                              XPAD, single_packet=False,
                                              queue_num=s)
                        else:
                            G = gpool.tile([P, KB, D], F16, tag="G")
                            nc.gpsimd.dma_gather(G[:], xs[s][:], it[:], NIDX,
                                                 NIDX, D, single_packet=False,
                                                 queue_num=s)
                        gts.append(G)
                    m4c = mpool.tile([P, KB * N_SHARDS * SLOTS],
                                     F8 if m4_fp8 else F16, tag="m4")
                    # ACT-ring HWDGE: keep the 1MB m4 loads out of the SP ring's
                    # FIFO so they never queue ahead of the small gather-idx DMAs
                    nc.scalar.dma_start(out=m4c[:], in_=m4_d[ch])
                    GRP = 8  # bins per PSUM bank (8 x 64 f32 = one 2KB bank)
                    for k in range(KB):
                        b = ch * KB + k
                        if k % GRP == 0:
                            pf = psf.tile([P, GRP * SLOTS], F32, tag="pf")
                        col = (k % GRP) * SLOTS
                        for s in range(N_SHARDS):
                            o = (k * N_SHARDS + s) * SLOTS
                            nc.tensor.matmul(
                                out=pf[:, col:col + SLOTS], lhsT=gts[s][:, k, :],
                                rhs=m4c[:, o:o + SLOTS],
                                start=(s == 0), stop=(s == N_SHARDS - 1))
                        if k % GRP == GRP - 1:
                            nc.scalar.copy(
                                out=featsT[:, (b - GRP + 1) * SLOTS:(b + 1) * SLOTS],
                                in_=pf[:])
                    # MLP chunks for the PREVIOUS gather-chunk (deps settled,
                    # so the in-order PE doesn't stall on the bin copies)
                    mlp_lo = (ch - 1) * KB * SLOTS // mlp_chunk if ch > 0 else 0
                    mlp_hi = ch * KB * SLOTS // mlp_chunk if ch > 0 else 0
                    if ch == nchunks - 1:
                        mlp_hi = nslots // mlp_chunk  # epilogue: last two chunks
                    for j in range(mlp_lo, mlp_hi):
                        cols = slice(j * mlp_chunk, (j + 1) * mlp_chunk)
                        ph = psh.tile([64, mlp_chunk], F32, tag="ph")
                        nc.tensor.matmul(out=ph[:], lhsT=w1_t[:], rhs=featsT[:, cols],
                                         start=True, stop=True)
                        hr = tpool.tile([64, mlp_chunk], F32, tag="hr")
                        nc.scalar.activation(out=hr[:], in_=ph[:], func=AF.Relu,
                                             bias=b1_t[:])
                        pa = psa.tile([8, mlp_chunk], F32, tag="pa")
                        nc.tensor.matmul(out=pa[:], lhsT=w2_t[:], rhs=hr[:],
                                         start=True, stop=True)
                        sg = tpool.tile([8, mlp_chunk], F32, tag="sg")
                        if m4_fp8:
                            # logits were computed from UNSCALED segment sums;
                            # b1==0 makes relu commute with the 1/count scale,
                            # so apply it here (before bias b2 / sigmoid)
                            pm = tpool.tile([8, mlp_chunk], F32, tag="pm")
                            nc.vector.scalar_tensor_tensor(
                                out=pm[:], in0=pa[:], scalar=0.0,
                                in1=icnt_t[:, cols], op0=OP.bypass,
                                op1=OP.mult)
                            sga = pm
                        else:
                            sga = pa
                        nc.scalar.activation(out=sg[:], in_=sga[:],
                                             func=AF.Sigmoid, bias=b2_t[:])
                        nc.vector.tensor_scalar(out=sg[:], in0=sg[:],
                                                scalar1=float(SIG_LO),
                                                scalar2=float(SIG_HI),
                                                op0=OP.max, op1=OP.min)
                        po = pso.tile([1, mlp_chunk], F32, tag="po")
                        nc.tensor.matmul(out=po[:], lhsT=mean_t[:], rhs=sg[:],
                                         start=True, stop=True)
                        nc.scalar.activation(out=out_sb[:, cols], in_=po[:],
                                             func=AF.Copy, bias=0.1)
            nc.sync.dma_start(out=out_d[:], in_=out_sb[:])
    nc.compile()
    return nc


# ---------------------------------------------------------------- entry point

def _host_fallback(out, segs, x, node_idx, hyperedge_idx, W1, b1, W2, b2):
    for s in segs:
        rows = x[node_idx[hyperedge_idx == s]]
        feats = rows.mean(axis=0) if len(rows) else np.zeros(IN_DIM, np.float32)
        h = np.maximum(np.einsum("d,hdk->hk", feats, W1) + b1, 0.0)
        alpha = np.einsum("hk,hk->h", h, W2) + b2
        w = 1.0 / (1.0 + np.exp(-np.clip(alpha, -5, 5)))
        out[s] = w.mean() * 0.9 + 0.1


def kernel(x, node_idx, hyperedge_idx, W1, b1, W2, b2):
    x = np.asarray(x, np.float32)
    node_idx = np.asarray(node_idx)
    hyperedge_idx = np.asarray(hyperedge_idx)
    W1 = np.asarray(W1, np.float32); b1 = np.asarray(b1, np.float32)
    W2 = np.asarray(W2, np.float32); b2 = np.asarray(b2, np.float32)

    packed = _pack(node_idx, hyperedge_idx)
    m = packed["meta"]
    consts = _make_mlp_consts(W1, b1, W2, b2)
    m4_fp8 = M4_FP8 and not np.any(b1)
    nc = build_nc(m["nbins"], m["nchunks"], N_CORES, m4_fp8=m4_fp8)
    in_maps = [_make_in_map(c, x, packed, consts, m4_fp8=m4_fp8)
               for c in range(N_CORES)]
    res = run_bass_kernel_spmd(nc, in_maps, list(range(N_CORES)))

    out = np.full(NUM_HYPEREDGES, np.nan, dtype=np.float32)
    om = packed["out_map"].reshape(N_CORES, -1)
    for c in range(N_CORES):
        core_out = res.results[c]["out"].reshape(-1)
        v = om[c] >= 0
        out[om[c][v]] = core_out[v]
    if len(packed["fallback"]):
        _host_fallback(out, packed["fallback"], x, node_idx, hyperedge_idx,
                       W1, b1, W2, b2)
    assert not np.isnan(out).any()
    return out

